# revision 10
# baseline (speedup 1.0000x reference)
"""Trainium2 Bass kernel for the pointer-network decoder (greedy decode).

Data-parallel over batch B=512 -> 8 cores x 64 rows. Each core runs the
full 128-step recurrent decode on its shard, fully on-chip:
  - LSTM + merge in h-major layout (hidden dim on partitions, 2 tiles),
  - glimpse/pointer additive attention with the [H, B*N] tanh tensors
    resident in SBUF; per-step query broadcast-add via stride-0
    DVE/GPSIMD tensor_tensor, tanh on ACT, v-dot as per-b matmuls into
    PSUM columns,
  - glimpse readout fused with the pointer query projection:
    F = ptr_Wq @ e_gl + ptr_bq stored l-major; q_ptr[:,b] = F[:,b,:].T @ p[b,:].T
  - log-softmax deferred: per-step shifted logits + sumexp staged to DRAM,
    final pass computes shifted - log(S) (ACT stays on one table set).
All arithmetic fp32 to track the reference argmax decisions.
"""
import sys
import numpy as np

sys.path.insert(0, '/opt/trn_rl_repo')

import concourse.bass as bass
import concourse.bacc as bacc
import concourse.mybir as mybir
from concourse.tile import TileContext
from concourse.bass_utils import run_bass_kernel_spmd
from concourse.masks import make_identity

FP = mybir.dt.float32
I32 = mybir.dt.int32
U8 = mybir.dt.uint8
AF = mybir.ActivationFunctionType
ALU = mybir.AluOpType
AX = mybir.AxisListType.X

B, N, E, H = 512, 128, 256, 256
COU = 11
C_TANH = 10.0
NCORES = 8
BL = B // NCORES          # 64 rows per core
HT = H // 128             # 2 h-tiles
F_RES = 32                # F' b-rows resident; BL - F_RES streamed per step
BCH = 8                   # b rows per tanh chunk (chunk free = BCH*N = 1024)
NEG_INF = float("-inf")


def ds(start, size):
    return bass.ds(start, size)


def _build(trace_scopes=False):
    nc = bacc.Bacc("TRN2", target_bir_lowering=False, debug=False)

    d = {}
    def inp(name, shape, dt=FP):
        d[name] = nc.dram_tensor(name, shape, dt, kind="ExternalInput")

    inp("dec0", [BL, E])
    inp("h0T", [H, BL])
    inp("c0T", [H, BL])
    inp("couT", [COU, BL])
    inp("mask0", [BL, N], U8)
    inp("ctxT", [H, BL, N])
    inp("emb", [N * BL, E])
    inp("Edist", [BL * N, N])
    inp("Vdm", [BL, N, N], U8)
    inp("WcombT", [E + H, 4 * H])
    inp("bias_act", [128, 8])      # col m = gate-row-chunk m bias (i/f/o pre-halved)
    inp("WmT0", [128, H])
    inp("WmT1", [128, H])
    inp("WmT2", [COU, H])
    inp("bm2", [128, HT])          # col m
    inp("WqglT", [H, H])           # sliced into 2 row-tiles on load
    inp("bqgl2", [128, HT])
    inp("WrefglT", [H, H])
    inp("brefgl2", [128, HT])
    inp("WrefptrT", [H, H])
    inp("brefptr2", [128, HT])
    inp("WqptrT", [H, H])
    inp("bqptr2", [128, HT])
    inp("vgl2", [128, HT])         # col kt
    inp("vptr2", [128, HT])

    lp_stage = nc.dram_tensor("lp_stage", [BL, N, N], FP)
    s_stage = nc.dram_tensor("s_stage", [BL, N], FP, kind="ExternalOutput")
    sels_out = nc.dram_tensor("sels_out", [BL, N], I32, kind="ExternalOutput")
    mask_out = nc.dram_tensor("mask_out", [BL, N], U8, kind="ExternalOutput")
    lp_out = nc.dram_tensor("lp_out", [BL, N, N], FP, kind="ExternalOutput")
    f_ext = nc.dram_tensor("f_ext", [BL - F_RES, 128, H], FP)

    with TileContext(nc) as tc:
        _body(nc, tc, d, lp_stage, s_stage, sels_out, mask_out, lp_out, f_ext)
    nc.finalize()
    return nc


def _body(nc, tc, d, lp_stage, s_stage, sels_out, mask_out, lp_out, f_ext):
    from contextlib import ExitStack
    ctx = ExitStack()
    with ctx:
        pers = ctx.enter_context(tc.tile_pool(name="pers", bufs=1))

        ident = pers.tile([128, 128], FP)
        make_identity(nc, ident[:])
        iota_l = pers.tile([BL, N], I32)
        nc.gpsimd.iota(iota_l[:], pattern=[[1, N]], base=0, channel_multiplier=0)
        biota128 = pers.tile([BL, 1], I32)
        nc.gpsimd.iota(biota128[:], pattern=[[0, 1]], base=0, channel_multiplier=N)
        biota1 = pers.tile([BL, 1], I32)
        nc.gpsimd.iota(biota1[:], pattern=[[0, 1]], base=0, channel_multiplier=1)
        neginf = pers.tile([BL, N], FP)
        nc.vector.memset(neginf[:], NEG_INF)
        iota_f = pers.tile([BL, N], FP)
        nc.vector.tensor_copy(out=iota_f[:], in_=iota_l[:])

        e_gl = [pers.tile([128, BL, N], FP, tag=f"e_gl{k}", name=f"e_gl{k}") for k in range(HT)]
        e_ptr = [pers.tile([128, BL, N], FP, tag=f"e_ptr{k}", name=f"e_ptr{k}") for k in range(HT)]
        f_res = pers.tile([128, F_RES, H], FP)

        def load2(name):   # [H, cols] dram -> two [128, cols] tiles
            t = [pers.tile([128, d[name].shape[1]], FP, tag=f"{name}{k}", name=f"{name}{k}") for k in range(HT)]
            for k in range(HT):
                nc.sync.dma_start(out=t[k][:], in_=d[name][k * 128:(k + 1) * 128, :])
            return t

        wqgl = load2("WqglT")
        wm0 = pers.tile([128, H], FP)
        nc.sync.dma_start(out=wm0[:], in_=d["WmT0"][:])
        wm1 = pers.tile([128, H], FP)
        nc.sync.dma_start(out=wm1[:], in_=d["WmT1"][:])
        wm2 = pers.tile([COU, H], FP)
        nc.sync.dma_start(out=wm2[:], in_=d["WmT2"][:])
        couT = pers.tile([COU, BL], FP)
        nc.sync.dma_start(out=couT[:], in_=d["couT"][:])

        def small(name):
            t = pers.tile([128, HT], FP, tag=name, name=f"sm_{name}")
            nc.sync.dma_start(out=t[:], in_=d[name][:])
            return t
        bias_act = pers.tile([128, 8], FP)
        nc.sync.dma_start(out=bias_act[:], in_=d["bias_act"][:])
        bm2 = small("bm2"); bqgl2 = small("bqgl2")
        vgl2 = small("vgl2"); vptr2 = small("vptr2")

        cT = [pers.tile([128, BL], FP, tag=f"cT{k}", name=f"cT{k}") for k in range(HT)]
        hT = [pers.tile([128, BL], FP, tag=f"hT{k}", name=f"hT{k}") for k in range(HT)]
        for k in range(HT):
            nc.sync.dma_start(out=cT[k][:], in_=d["c0T"][k * 128:(k + 1) * 128, :])
            nc.sync.dma_start(out=hT[k][:], in_=d["h0T"][k * 128:(k + 1) * 128, :])
        x_sb = pers.tile([BL, E], FP)
        nc.sync.dma_start(out=x_sb[:], in_=d["dec0"][:])
        mask_cur = pers.tile([BL, N], U8)
        nc.sync.dma_start(out=mask_cur[:], in_=d["mask0"][:])
        knn_oh = pers.tile([BL, N], U8)
        nc.vector.memset(knn_oh[:], 0)
        idx = pers.tile([BL, 1], I32)
        nc.vector.memset(idx[:], 0)
        idxf = pers.tile([BL, 1], FP)
        nc.vector.memset(idxf[:], 0.0)

        # ---------------- precompute ----------------
        with tc.tile_pool(name="pre", bufs=2) as pre, \
             tc.tile_pool(name="pre1", bufs=1) as pre1, \
             tc.tile_pool(name="pre_ps", bufs=2, space="PSUM") as pre_ps:
            wqptr = [pre1.tile([128, H], FP, tag=f"wqp{k}", name=f"wqp{k}") for k in range(HT)]
            for k in range(HT):
                nc.sync.dma_start(out=wqptr[k][:], in_=d["WqptrT"][k * 128:(k + 1) * 128, :])
            wrefgl = [pre1.tile([128, H], FP, tag=f"wrg{k}", name=f"wrg{k}") for k in range(HT)]
            wrefptr = [pre1.tile([128, H], FP, tag=f"wrp{k}", name=f"wrp{k}") for k in range(HT)]
            for k in range(HT):
                nc.sync.dma_start(out=wrefgl[k][:], in_=d["WrefglT"][k * 128:(k + 1) * 128, :])
                nc.sync.dma_start(out=wrefptr[k][:], in_=d["WrefptrT"][k * 128:(k + 1) * 128, :])
            brefgl2 = pre1.tile([128, HT], FP, tag="brg")
            nc.sync.dma_start(out=brefgl2[:], in_=d["brefgl2"][:])
            brefptr2 = pre1.tile([128, HT], FP, tag="brp")
            nc.sync.dma_start(out=brefptr2[:], in_=d["brefptr2"][:])
            bqptr2 = pre1.tile([128, HT], FP, tag="bqp")
            nc.sync.dma_start(out=bqptr2[:], in_=d["bqptr2"][:])

            NCH = (BL * N) // 512
            ctx_flat = d["ctxT"][:].rearrange("h b l -> h (b l)")
            for ch in range(NCH):
                cs = slice(ch * 512, (ch + 1) * 512)
                cch = [pre.tile([128, 512], FP, tag=f"ctx{kt}", name=f"cch{kt}") for kt in range(HT)]
                for kt in range(HT):
                    nc.sync.dma_start(out=cch[kt][:], in_=ctx_flat[kt * 128:(kt + 1) * 128, cs])
                for oc in range(HT):
                    ps_gl = pre_ps.tile([128, 512], FP, tag="psgl")
                    ps_pt = pre_ps.tile([128, 512], FP, tag="pspt")
                    for kt in range(HT):
                        nc.tensor.matmul(out=ps_gl[:], lhsT=wrefgl[kt][:, oc * 128:(oc + 1) * 128],
                                         rhs=cch[kt][:], start=(kt == 0), stop=(kt == HT - 1))
                        nc.tensor.matmul(out=ps_pt[:], lhsT=wrefptr[kt][:, oc * 128:(oc + 1) * 128],
                                         rhs=cch[kt][:], start=(kt == 0), stop=(kt == HT - 1))
                    nc.vector.tensor_scalar(
                        out=e_gl[oc][:].rearrange("h b l -> h (b l)")[:, cs],
                        in0=ps_gl[:], scalar1=brefgl2[:, oc:oc + 1], scalar2=None, op0=ALU.add)
                    nc.vector.tensor_scalar(
                        out=e_ptr[oc][:].rearrange("h b l -> h (b l)")[:, cs],
                        in0=ps_pt[:], scalar1=brefptr2[:, oc:oc + 1], scalar2=None, op0=ALU.add)

            for b in range(BL):
                for oc in range(HT):
                    ps_f = pre_ps.tile([128, 128], FP, tag="psf")
                    for kt in range(HT):
                        nc.tensor.matmul(out=ps_f[:], lhsT=wqptr[kt][:, oc * 128:(oc + 1) * 128],
                                         rhs=e_gl[kt][:, b, :], start=(kt == 0), stop=(kt == HT - 1))
                    fh = pre.tile([128, 128], FP, tag="fh")
                    nc.vector.tensor_scalar(out=fh[:], in0=ps_f[:],
                                            scalar1=bqptr2[:, oc:oc + 1], scalar2=None, op0=ALU.add)
                    ps_ft = pre_ps.tile([128, 128], FP, tag="psft")
                    nc.tensor.transpose(out=ps_ft[:], in_=fh[:], identity=ident[:])
                    if b < F_RES:
                        nc.vector.tensor_copy(out=f_res[:, b, oc * 128:(oc + 1) * 128], in_=ps_ft[:])
                    else:
                        ft = pre.tile([128, 128], FP, tag="ft")
                        nc.vector.tensor_copy(out=ft[:], in_=ps_ft[:])
                        nc.sync.dma_start(out=f_ext[b - F_RES, :, oc * 128:(oc + 1) * 128], in_=ft[:])

        # ---------------- decode loop ----------------
        loop_sb = ctx.enter_context(tc.tile_pool(name="loop_sb", bufs=2))
        loop_s = ctx.enter_context(tc.tile_pool(name="loop_s", bufs=2))
        loop_w = ctx.enter_context(tc.tile_pool(name="loop_w", bufs=2))
        loop_f = ctx.enter_context(tc.tile_pool(name="loop_f", bufs=4))
        work = ctx.enter_context(tc.tile_pool(name="work", bufs=1))
        wps1 = ctx.enter_context(tc.tile_pool(name="wps1", bufs=1, space="PSUM"))
        wps2 = ctx.enter_context(tc.tile_pool(name="wps2", bufs=2, space="PSUM"))

        wcomb_m = d["WcombT"][:]  # [512, 1024]

        def step_body(i, first):
            # A: scatter prev pick into mask, then mask_modify
            if not first:
                oh = work.tile([BL, N], U8, tag="oh")
                nc.vector.tensor_scalar(out=oh[:], in0=iota_f[:], scalar1=idxf[:, :1],
                                        scalar2=None, op0=ALU.is_equal)
                nc.vector.tensor_tensor(out=mask_cur[:], in0=mask_cur[:], in1=oh[:],
                                        op=ALU.bitwise_or)
            allt = work.tile([BL, 1], U8, tag="allt")
            nc.vector.tensor_reduce(out=allt[:], in_=mask_cur[:], axis=AX, op=ALU.min)
            nall = work.tile([BL, 1], U8, tag="nall")
            nc.vector.tensor_scalar(out=nall[:], in0=allt[:], scalar1=1, scalar2=None,
                                    op0=ALU.is_lt)
            nc.vector.tensor_scalar(out=mask_cur[:, N - 1:N], in0=mask_cur[:, N - 1:N],
                                    scalar1=nall[:, :1], scalar2=None, op0=ALU.bitwise_and)

            # B: full mask = mask | Vdm_row | knn_oh
            vrow = loop_sb.tile([BL, N], U8, tag="vrow")
            nc.sync.dma_start(out=vrow[:], in_=(d["Vdm"][:, 0, :] if first
                                                else d["Vdm"][:, ds(i, 1), :]))
            fmask = work.tile([BL, N], U8, tag="fmask")
            nc.vector.tensor_tensor(out=fmask[:], in0=vrow[:], in1=knn_oh[:], op=ALU.bitwise_or)
            nc.vector.tensor_tensor(out=fmask[:], in0=fmask[:], in1=mask_cur[:], op=ALU.bitwise_or)

            # C: LSTM
            xT = []
            for kt in range(HT):
                psx = wps2.tile([128, BL], FP, tag="pmm")
                nc.tensor.transpose(out=psx[:], in_=x_sb[:, kt * 128:(kt + 1) * 128],
                                    identity=ident[:BL, :BL])
                t = work.tile([128, BL], FP, tag=f"xT{kt}", name=f"xTt{kt}")
                nc.vector.tensor_copy(out=t[:], in_=psx[:])
                xT.append(t)
            rhs_all = [xT[0][:], xT[1][:], hT[0][:], hT[1][:]]
            gates_ps = wps1.tile([128, 8 * BL], FP, tag="gates")
            for m in range(8):
                wt = loop_w.tile([128, 4, 128], FP, tag="wst")
                nc.sync.dma_start(
                    out=wt[:],
                    in_=wcomb_m[:, m * 128:(m + 1) * 128].rearrange("(k p) c -> p k c", p=128))
                for kt in range(4):
                    nc.tensor.matmul(out=gates_ps[:, m * BL:(m + 1) * BL],
                                     lhsT=wt[:, kt, :], rhs=rhs_all[kt],
                                     start=(kt == 0), stop=(kt == 3))
            sig = {}
            for gi, gname in enumerate(("i", "f", "g", "o")):
                for t in range(HT):
                    m = gi * HT + t
                    tg = work.tile([128, BL], FP, tag=f"tg{gname}{t}", name=f"tg{gname}{t}")
                    nc.scalar.activation(out=tg[:], in_=gates_ps[:, m * BL:(m + 1) * BL],
                                         func=AF.Tanh, bias=bias_act[:, m:m + 1],
                                         scale=(1.0 if gname == "g" else 0.5))
                    if gname != "g":
                        nc.vector.tensor_scalar(out=tg[:], in0=tg[:], scalar1=0.5,
                                                scalar2=0.5, op0=ALU.mult, op1=ALU.add)
                    sig[(gname, t)] = tg
            for t in range(HT):
                t1 = work.tile([128, BL], FP, tag=f"t1{t}", name=f"t1{t}")
                nc.vector.tensor_tensor(out=t1[:], in0=sig[("f", t)][:], in1=cT[t][:], op=ALU.mult)
                t2 = work.tile([128, BL], FP, tag=f"t2{t}", name=f"t2{t}")
                nc.vector.tensor_tensor(out=t2[:], in0=sig[("i", t)][:], in1=sig[("g", t)][:], op=ALU.mult)
                nc.vector.tensor_tensor(out=cT[t][:], in0=t1[:], in1=t2[:], op=ALU.add)
                tct = work.tile([128, BL], FP, tag=f"tct{t}", name=f"tct{t}")
                nc.scalar.activation(out=tct[:], in_=cT[t][:], func=AF.Tanh)
                nc.vector.tensor_tensor(out=hT[t][:], in0=sig[("o", t)][:], in1=tct[:], op=ALU.mult)

            # D: merge -> g0T ; q_gl
            g0T = [work.tile([128, BL], FP, tag=f"g0T{k}", name=f"g0T{k}") for k in range(HT)]
            for m in range(HT):
                psm = wps2.tile([128, BL], FP, tag="pmm")
                nc.tensor.matmul(out=psm[:], lhsT=wm0[:, m * 128:(m + 1) * 128],
                                 rhs=hT[0][:], start=True, stop=False)
                nc.tensor.matmul(out=psm[:], lhsT=wm1[:, m * 128:(m + 1) * 128],
                                 rhs=hT[1][:], start=False, stop=False)
                nc.tensor.matmul(out=psm[:], lhsT=wm2[:, m * 128:(m + 1) * 128],
                                 rhs=couT[:], start=False, stop=True)
                nc.vector.tensor_scalar(out=g0T[m][:], in0=psm[:], scalar1=bm2[:, m:m + 1],
                                        scalar2=None, op0=ALU.add)
            qgl = [work.tile([128, BL], FP, tag=f"qgl{k}", name=f"qglt{k}") for k in range(HT)]
            for m in range(HT):
                psq = wps2.tile([128, BL], FP, tag="pmm")
                for kt in range(HT):
                    nc.tensor.matmul(out=psq[:], lhsT=wqgl[kt][:, m * 128:(m + 1) * 128],
                                     rhs=g0T[kt][:], start=(kt == 0), stop=(kt == HT - 1))
                nc.vector.tensor_scalar(out=qgl[m][:], in0=psq[:], scalar1=bqgl2[:, m:m + 1],
                                        scalar2=None, op0=ALU.add)

            # attention core
            def attn(e_t, q_t, v2, vcol_by_kt, uname):
                uT_ps = [wps1.tile([128, BL], FP, tag=f"uT{kt}", name=f"uTp{kt}")
                         for kt in range(HT)]
                for kt in range(HT):
                    for c in range(BL // BCH):
                        bs = slice(c * BCH, (c + 1) * BCH)
                        s_ch = loop_s.tile([128, BCH, N], FP, tag="s_ch")
                        eng = nc.vector if (c % 2 == 0) else nc.gpsimd
                        eng.tensor_tensor(out=s_ch[:], in0=e_t[kt][:, bs, :],
                                          in1=q_t[kt][:, bs].to_broadcast([128, BCH, N]),
                                          op=ALU.add)
                        nc.scalar.activation(out=s_ch[:], in_=s_ch[:], func=AF.Tanh)
                        for j in range(BCH):
                            b = c * BCH + j
                            nc.tensor.matmul(out=uT_ps[kt][:, b:b + 1], lhsT=s_ch[:, j, :],
                                             rhs=v2[:, kt:kt + 1],
                                             start=True, stop=True)
                uT_sb = work.tile([128, BL], FP, tag=f"uTsb{uname}", name=f"uTsb{uname}")
                nc.vector.tensor_copy(out=uT_sb[:], in_=uT_ps[0][:])
                nc.vector.tensor_tensor(out=uT_sb[:], in0=uT_sb[:], in1=uT_ps[1][:], op=ALU.add)
                u_ps = wps2.tile([BL, 128], FP, tag="pmm")
                nc.tensor.transpose(out=u_ps[:], in_=uT_sb[:], identity=ident[:])
                u_sb = work.tile([BL, N], FP, tag=f"usb{uname}", name=f"usb{uname}")
                nc.vector.tensor_copy(out=u_sb[:], in_=u_ps[:])
                return u_sb

            u_gl = attn(e_gl, qgl, vgl2, 0, "gl")

            # G: glimpse softmax -> p ; fused readout/q_ptr
            gsel = work.tile([BL, N], FP, tag="gsel")
            nc.vector.select(out=gsel[:], mask=fmask[:], on_true=neginf[:], on_false=u_gl[:])
            gmx = work.tile([BL, 1], FP, tag="gmx")
            nc.vector.reduce_max(out=gmx[:], in_=gsel[:], axis=AX)
            gsh = work.tile([BL, N], FP, tag="gsh")
            nc.vector.tensor_scalar(out=gsh[:], in0=gsel[:], scalar1=gmx[:, :1],
                                    scalar2=None, op0=ALU.subtract)
            gex = work.tile([BL, N], FP, tag="gex")
            nc.scalar.activation(out=gex[:], in_=gsh[:], func=AF.Exp)
            gsm = work.tile([BL, 1], FP, tag="gsm")
            nc.vector.reduce_sum(out=gsm[:], in_=gex[:], axis=AX)
            grec = work.tile([BL, 1], FP, tag="grec")
            nc.vector.reciprocal(out=grec[:], in_=gsm[:])
            gp = work.tile([BL, N], FP, tag="gp")
            nc.vector.tensor_scalar(out=gp[:], in0=gex[:], scalar1=grec[:, :1],
                                    scalar2=None, op0=ALU.mult)
            pT_ps = wps2.tile([128, BL], FP, tag="pmm")
            nc.tensor.transpose(out=pT_ps[:], in_=gp[:], identity=ident[:BL, :BL])
            pT = work.tile([128, BL], FP, tag="pT")
            nc.vector.tensor_copy(out=pT[:], in_=pT_ps[:])

            qptr_ps = wps1.tile([128, HT * BL], FP, tag="qptr")
            for b in range(BL):
                if b < F_RES:
                    for oc in range(HT):
                        nc.tensor.matmul(out=qptr_ps[:, oc * BL + b:oc * BL + b + 1],
                                         lhsT=f_res[:, b, oc * 128:(oc + 1) * 128],
                                         rhs=pT[:, b:b + 1], start=True, stop=True)
                else:
                    fstr = loop_f.tile([128, H], FP, tag="fstr")
                    nc.sync.dma_start(out=fstr[:], in_=f_ext[b - F_RES, :, :])
                    for oc in range(HT):
                        nc.tensor.matmul(out=qptr_ps[:, oc * BL + b:oc * BL + b + 1],
                                         lhsT=fstr[:, oc * 128:(oc + 1) * 128],
                                         rhs=pT[:, b:b + 1], start=True, stop=True)
            qptr = [work.tile([128, BL], FP, tag=f"qptr{k}", name=f"qptrt{k}") for k in range(HT)]
            for oc in range(HT):
                nc.vector.tensor_copy(out=qptr[oc][:], in_=qptr_ps[:, oc * BL:(oc + 1) * BL])

            # H: pointer attention + logits + argmax
            u_pt = attn(e_ptr, qptr, vptr2, 0, "pt")
            tnh = work.tile([BL, N], FP, tag="tnh")
            nc.scalar.activation(out=tnh[:], in_=u_pt[:], func=AF.Tanh)
            logit = work.tile([BL, N], FP, tag="logit")
            nc.vector.tensor_scalar(out=logit[:], in0=tnh[:], scalar1=C_TANH,
                                    scalar2=None, op0=ALU.mult)
            psel = work.tile([BL, N], FP, tag="psel")
            nc.vector.select(out=psel[:], mask=fmask[:], on_true=neginf[:], on_false=logit[:])
            pmx = work.tile([BL, 1], FP, tag="pmx")
            nc.vector.reduce_max(out=pmx[:], in_=psel[:], axis=AX)
            psh = work.tile([BL, N], FP, tag="psh")
            nc.vector.tensor_scalar(out=psh[:], in0=psel[:], scalar1=pmx[:, :1],
                                    scalar2=None, op0=ALU.subtract)
            pex = work.tile([BL, N], FP, tag="pex")
            nc.scalar.activation(out=pex[:], in_=psh[:], func=AF.Exp)
            psm2 = work.tile([BL, 1], FP, tag="psm2")
            nc.vector.reduce_sum(out=psm2[:], in_=pex[:], axis=AX)
            nc.sync.dma_start(out=(lp_stage[:, 0, :] if first else lp_stage[:, ds(i, 1), :]),
                              in_=psh[:])
            nc.sync.dma_start(out=(s_stage[:, 0:1] if first else s_stage[:, ds(i, 1)]),
                              in_=psm2[:])

            peq = work.tile([BL, N], FP, tag="peq")
            nc.vector.tensor_scalar(out=peq[:], in0=psh[:], scalar1=0.0, scalar2=None,
                                    op0=ALU.is_ge)
            pcand = work.tile([BL, N], FP, tag="pcand")
            nc.vector.scalar_tensor_tensor(out=pcand[:], in0=peq[:], scalar=-512.0,
                                           in1=iota_f[:], op0=ALU.mult, op1=ALU.add)
            pmin = work.tile([BL, 1], FP, tag="pmin")
            nc.vector.tensor_reduce(out=pmin[:], in_=pcand[:], axis=AX, op=ALU.min)
            nc.vector.tensor_scalar(out=idxf[:], in0=pmin[:], scalar1=512.0, scalar2=None,
                                    op0=ALU.add)
            nc.vector.tensor_copy(out=idx[:], in_=idxf[:])
            nc.sync.dma_start(out=(sels_out[:, 0:1] if first else sels_out[:, ds(i, 1)]),
                              in_=idx[:])

            # I: knn mask + gathers for next step
            eidx = work.tile([BL, 1], I32, tag="eidx")
            nc.vector.tensor_tensor(out=eidx[:], in0=idx[:], in1=biota128[:], op=ALU.add)
            erow = work.tile([BL, N], FP, tag="erow")
            nc.gpsimd.indirect_dma_start(
                out=erow[:], out_offset=None, in_=d["Edist"][:],
                in_offset=bass.IndirectOffsetOnAxis(ap=eidx[:, :1], axis=0))
            unv = work.tile([BL, N], FP, tag="unv")
            nc.vector.tensor_scalar(out=unv[:], in0=mask_cur[:], scalar1=1, scalar2=None,
                                    op0=ALU.is_lt)
            nsum = work.tile([BL, 1], FP, tag="nsum")
            nc.vector.reduce_sum(out=nsum[:], in_=unv[:], axis=AX)
            valid = work.tile([BL, 1], FP, tag="valid")
            nc.vector.tensor_scalar(out=valid[:], in0=nsum[:], scalar1=3.0, scalar2=None,
                                    op0=ALU.is_ge)
            ei = work.tile([BL, N], FP, tag="ei")
            nc.vector.tensor_tensor(out=ei[:], in0=erow[:], in1=unv[:], op=ALU.mult)
            kmx = work.tile([BL, 1], FP, tag="kmx")
            nc.vector.reduce_max(out=kmx[:], in_=ei[:], axis=AX)
            keq = work.tile([BL, N], FP, tag="keq")
            nc.vector.tensor_scalar(out=keq[:], in0=ei[:], scalar1=kmx[:, :1], scalar2=None,
                                    op0=ALU.is_ge)
            kcand = work.tile([BL, N], FP, tag="kcand")
            nc.vector.scalar_tensor_tensor(out=kcand[:], in0=keq[:], scalar=-512.0,
                                           in1=iota_f[:], op0=ALU.mult, op1=ALU.add)
            kmin = work.tile([BL, 1], FP, tag="kmin")
            nc.vector.tensor_reduce(out=kmin[:], in_=kcand[:], axis=AX, op=ALU.min)
            kidx = work.tile([BL, 1], FP, tag="kidx")
            nc.vector.tensor_scalar(out=kidx[:], in0=kmin[:], scalar1=512.0, scalar2=None,
                                    op0=ALU.add)
            koh = work.tile([BL, N], FP, tag="koh")
            nc.vector.tensor_scalar(out=koh[:], in0=iota_f[:], scalar1=kidx[:, :1],
                                    scalar2=None, op0=ALU.is_equal)
            nc.vector.tensor_scalar(out=knn_oh[:], in0=koh[:], scalar1=valid[:, :1],
                                    scalar2=None, op0=ALU.mult)

            xidx = work.tile([BL, 1], I32, tag="xidx")
            nc.vector.scalar_tensor_tensor(out=xidx[:], in0=idx[:], scalar=BL,
                                           in1=biota1[:], op0=ALU.mult, op1=ALU.add)
            nc.gpsimd.indirect_dma_start(
                out=x_sb[:], out_offset=None, in_=d["emb"][:],
                in_offset=bass.IndirectOffsetOnAxis(ap=xidx[:, :1], axis=0))

        step_body(0, True)
        with tc.For_i(1, N, hint_engines=(mybir.EngineType.PE,)) as iv:
            step_body(iv, False)

        nc.sync.dma_start(out=mask_out[:], in_=mask_cur[:])

        # finale: lp = shifted - log(S)
        s_read = pers.tile([BL, N], FP, tag="s_read")
        nc.sync.dma_start(out=s_read[:], in_=s_stage[:])
        lns = pers.tile([BL, N], FP, tag="lns")
        nc.scalar.activation(out=lns[:], in_=s_read[:], func=AF.Ln)
        with tc.tile_pool(name="fin", bufs=2) as fin:
            for chx in range(32):
                st = slice(chx * 4, (chx + 1) * 4)
                lpc = fin.tile([BL, 4, N], FP, tag="lpc")
                nc.sync.dma_start(out=lpc[:], in_=lp_stage[:, st, :])
                nc.vector.tensor_tensor(out=lpc[:], in0=lpc[:],
                                        in1=lns[:, st].to_broadcast([BL, 4, N]),
                                        op=ALU.subtract)
                nc.sync.dma_start(out=lp_out[:, st, :], in_=lpc[:])


_CACHED = {}


def _get_nc():
    if "nc" not in _CACHED:
        _CACHED["nc"] = _build()
    return _CACHED["nc"]


def prep_inmaps(decoder_input, embedded_inputs, h0, c0, context, V_reach_mask_t,
                embed_cou, V_decode_mask, batch_masked_E,
                W_ih, W_hh, b_ih, b_hh, W_merge, b_merge,
                gl_Wq, gl_bq, gl_Wref, gl_bref, gl_v,
                ptr_Wq, ptr_bq, ptr_Wref, ptr_bref, ptr_v):
    f32 = np.float32

    def T2(v):  # [256] -> [128, 2], col k = rows k*128:(k+1)*128
        return np.ascontiguousarray(np.asarray(v, f32).reshape(HT, 128).T)

    WcombT = np.ascontiguousarray(np.concatenate(
        [np.asarray(W_ih), np.asarray(W_hh)], axis=1).T.astype(f32))
    bias = (np.asarray(b_ih) + np.asarray(b_hh)).astype(f32)
    scl = np.full(4 * H, 0.5, f32)
    scl[2 * H:3 * H] = 1.0  # g gate keeps full bias
    bias_act = np.ascontiguousarray((bias * scl).reshape(8, 128).T)
    WmT = np.asarray(W_merge, f32).T  # [267, 256]
    shared = dict(
        WcombT=WcombT, bias_act=bias_act,
        WmT0=np.ascontiguousarray(WmT[0:128]),
        WmT1=np.ascontiguousarray(WmT[128:256]),
        WmT2=np.ascontiguousarray(WmT[256:256 + COU]),
        bm2=T2(b_merge),
        WqglT=np.ascontiguousarray(np.asarray(gl_Wq).T.astype(f32)),
        bqgl2=T2(gl_bq),
        WrefglT=np.ascontiguousarray(np.asarray(gl_Wref).T.astype(f32)),
        brefgl2=T2(gl_bref),
        WrefptrT=np.ascontiguousarray(np.asarray(ptr_Wref).T.astype(f32)),
        brefptr2=T2(ptr_bref),
        WqptrT=np.ascontiguousarray(np.asarray(ptr_Wq).T.astype(f32)),
        bqptr2=T2(ptr_bq),
        vgl2=T2(gl_v), vptr2=T2(ptr_v),
    )

    in_maps = []
    for c in range(NCORES):
        bs = slice(c * BL, (c + 1) * BL)
        m = dict(shared)
        m["dec0"] = np.ascontiguousarray(np.asarray(decoder_input)[bs].astype(f32))
        m["h0T"] = np.ascontiguousarray(np.asarray(h0)[bs].T.astype(f32))
        m["c0T"] = np.ascontiguousarray(np.asarray(c0)[bs].T.astype(f32))
        m["couT"] = np.ascontiguousarray(np.asarray(embed_cou)[bs].T.astype(f32))
        m["mask0"] = np.ascontiguousarray(np.asarray(V_reach_mask_t)[bs].astype(np.uint8))
        m["ctxT"] = np.ascontiguousarray(
            np.asarray(context)[:, bs, :].transpose(2, 1, 0).astype(f32))
        m["emb"] = np.ascontiguousarray(
            np.asarray(embedded_inputs)[:, bs, :].reshape(N * BL, E).astype(f32))
        m["Edist"] = np.ascontiguousarray(
            np.asarray(batch_masked_E)[bs].reshape(BL * N, N).astype(f32))
        m["Vdm"] = np.ascontiguousarray(np.asarray(V_decode_mask)[bs].astype(np.uint8))
        in_maps.append(m)
    return in_maps


def kernel(**inputs):
    nc = _get_nc()
    in_maps = prep_inmaps(**inputs)
    res = run_bass_kernel_spmd(nc, in_maps, core_ids=list(range(NCORES)))
    _CACHED["last_results"] = res

    lp = np.concatenate([r["lp_out"] for r in res.results], axis=0)
    sels = np.concatenate([r["sels_out"] for r in res.results], axis=0).astype(np.int32)
    mask = np.concatenate([r["mask_out"] for r in res.results], axis=0).astype(bool)
    return lp, sels, mask
